# revision 53
# baseline (speedup 1.0000x reference)
"""Trainium2 Bass kernel for empirical CRPS loss (mean reduction), v7.

All inputs are pre-rounded to f16 on the host (pure dtype cast -- rounding
commutes with min/max so results are identical to casting on device, and it
halves HBM traffic and eliminates the DVE convert passes).

Two concurrent pipelines per core (pure data parallel over pixels, 8 cores):

SORT path (pixels-on-free, [128, FB] f16 blocks), for SORT_PX pixels:
  per-pixel pairwise term Sum_{i<j}|x_i-x_j| = Sum_k (2k-19) X_(k) via a
  93-comparator sorting network on DVE (f16 2x mode, one DMA per tile,
  strided multi-block fusion), then the antisymmetric pairing
  D_j = X_(19-j) - X_(j) (10 DVE subs), prescale by w_j = 19-2j (DVE
  tensor_scalar, 4x mode), a DVE pre-reduce (10 -> 5 blocks), and ONE
  ScalarE accumulate -> 1 column of per-partition partial sums per tile.

PE path (samples-on-partitions [126, cols]): host pre-packs pe_x so rows
  20g+i = sample i of pixel-group g (6 groups), rows 120+g = target y.
  - ALL pixels: |x_i - y| term: 1 matmul pass (120 pair rows) per 512-col
    slice into PSUM + ScalarE Abs (in place on PSUM) + accum.
  - The last FULL_CHUNKS chunks additionally run 9 more passes covering all
    190 x-pairs per group (1140 rows) -> full pairwise term on PE+ACT
    instead of the DVE sort.
  Host separates x-pair vs y-pair contributions by pass column.

Scheduling: a small warmup sort tile first (DVE starts ~4us in); each
round's PE chunks are emitted BETWEEN network layers, placed proportionally
to their ScalarE cost with a half-step-early bias, so ScalarE never starves
while the network runs.

Host combine (f64): CRPS = (Y/N - (PW_sort + PW_pe)/N^2) / P_TOTAL.
"""
import numpy as np

N = 20
P_TOTAL = 4 * 1 * 12 * 256 * 256   # 3145728
N_CORES = 8
P_CORE = P_TOTAL // N_CORES        # 393216

G = 6                  # pixel groups in PE layout
CHUNK_COLS = 2048      # pe_x columns per chunk
CHUNK_PX = G * CHUNK_COLS          # 12288 px per chunk
TOTAL_CHUNKS = P_CORE // CHUNK_PX  # 32
FULL_CHUNKS = 10       # chunks whose pairwise term runs on PE+ACT
SORT_PX = P_CORE - FULL_CHUNKS * CHUNK_PX
PE_COLS = P_CORE // G  # 65536
NPASS = 10             # pass 0 = y pairs, 1..9 = x pairs
MMF = 512              # matmul free columns (one PSUM bank)

# sort tiling: SORT_PX/128 columns split into tiles; a small warmup tile
# first so DVE starts within ~4us instead of waiting on a 6MB DMA
SORT_FBS = [192, 960, 960]
assert sum(SORT_FBS) * 128 == SORT_PX
ND = 10                # D blocks

N_SORT_COLS = len(SORT_FBS)
N_FULL_COLS = FULL_CHUNKS * NPASS
N_Y_COLS = TOTAL_CHUNKS - FULL_CHUNKS
NCOLS = N_SORT_COLS + N_FULL_COLS + N_Y_COLS
NCOLS_PAD = ((NCOLS + 15) // 16) * 16

PAIRS = [(i, j) for i in range(N) for j in range(i + 1, N)]  # 190

_CACHE = {}

# --- sorting network (93 CEs): two optimal 29-CE 10-sorters + Batcher merge
SORT10 = [(4, 9), (3, 8), (2, 7), (1, 6), (0, 5),
          (1, 4), (6, 9), (0, 3), (5, 8),
          (0, 2), (3, 6), (7, 9),
          (0, 1), (2, 4), (5, 7), (8, 9),
          (1, 2), (4, 6), (7, 8), (3, 5),
          (2, 5), (6, 8), (1, 3), (4, 7),
          (2, 3), (6, 7),
          (3, 4), (5, 6),
          (4, 5)]


def _oe_merge(a, b, net):
    n, m = len(a), len(b)
    if n == 0 or m == 0:
        return
    if n == 1 and m == 1:
        net.append((a[0], b[0]))
        return
    _oe_merge(a[::2], b[::2], net)
    _oe_merge(a[1::2], b[1::2], net)
    c = list(a) + list(b)
    for i in range(1, n + m - 1, 2):
        net.append((c[i], c[i + 1]))


def sorting_network(n=N):
    assert n == 20
    net = [(i, j) for (i, j) in SORT10]
    net += [(i + 10, j + 10) for (i, j) in SORT10]
    _oe_merge(list(range(10)), list(range(10, 20)), net)
    return net


def build_wmat():
    w = np.zeros((126, NPASS * 128), np.float16)
    for g in range(G):                       # pass 0: y pairs
        for i in range(N):
            r = g * N + i
            w[20 * g + i, r] = 1.0
            w[120 + g, r] = -1.0
    rows = []
    for g in range(G):                       # passes 1..9: x pairs
        for (i, j) in PAIRS:
            rows.append((20 * g + i, 20 * g + j))
    for r, (pi, pj) in enumerate(rows):
        w[pi, 128 + r] = 1.0
        w[pj, 128 + r] = -1.0
    return w


def _emit_sort_tile_load(nc, pools, fsort, p0, fb):
    """One f16 DMA of all 20 sample blocks into buf; returns buf, slot.
    Inputs arrive pre-rounded to f16 from the host (rounding commutes with
    min/max, so the sorted values are bit-identical to sorting f32 then
    rounding)."""
    import concourse.mybir as mybir
    from concourse.ap import AP

    F16 = mybir.dt.float16
    wkp, dpp = pools
    NBUF = N + 4
    mfb = max(SORT_FBS)

    buf = wkp.tile([128, NBUF * mfb], F16, tag="buf")
    slot = list(range(N))

    src = AP(fsort[:, :].tensor, p0,
             [[fb, 128], [SORT_PX, N], [1, fb]])
    nc.sync.dma_start(
        buf[:, :N * fb].rearrange("p (n f) -> p n f", n=N), src)
    return buf, slot


def _emit_sort_tile_sort(nc, pools, buf, slot, acc, acc_col, fb,
                         chunk_emitters=()):
    """DVE sorting network (minus first layer) + D blocks + prescale +
    ScalarE colsum. `slot` holds the wire->block map from the fused
    first-layer pass. `chunk_emitters` are PE-chunk closures emitted
    between network layers so their casts land inside the DVE stream
    (keeps ScalarE fed while the network runs)."""
    import concourse.mybir as mybir
    from concourse.ap import AP

    F16 = mybir.dt.float16
    wkp, dpp = pools
    net = sorting_network(N)

    dtile = dpp.tile([128, ND * max(SORT_FBS)], F16, tag="d")

    # sort the 20 blocks (max -> free slot, min -> in place), fusing
    # wire-disjoint comparators with arithmetic-progression slots.
    frees = [N + k for k in range(4)]

    def ap2(base_slot, step, cnt):
        bap = buf[:, base_slot * fb:(base_slot + 1) * fb]
        if cnt == 1:
            return bap
        return AP(bap.tensor, bap.offset,
                  [list(bap.ap[0]), [step * fb, cnt], list(bap.ap[1])])

    layers, cur, used = [], [], set()
    for (i, j) in net:
        if i in used or j in used:
            layers.append(cur)
            cur, used = [], set()
        cur.append((i, j))
        used.update((i, j))
    layers.append(cur)

    # place chunk emissions across the network proportionally to their
    # ScalarE work (full chunk ~10 passes, y chunk 1) so ACT never starves
    chunk_emitters = list(chunk_emitters)
    nlay = len(layers)
    placement = []
    if chunk_emitters:
        tot_w = sum(w for _, w in chunk_emitters)
        cum = 0
        for em, w in chunk_emitters:
            # half-step early so ScalarE always has backlog
            lay = max(0, (2 * cum - w) * nlay // (2 * tot_w))
            placement.append((min(nlay - 1, lay), em))
            cum += w
    emitted = 0

    for li, layer in enumerate(layers):
        while emitted < len(placement) and placement[emitted][0] <= li:
            placement[emitted][1]()
            emitted += 1
        groups = []
        for (i, j) in layer:
            si, sj = slot[i], slot[j]
            g = groups[-1] if groups else None
            if g is not None and len(g) >= 1:
                if len(g) == 1:
                    g.append((i, j, si, sj))
                    continue
                di, dj = g[1][2] - g[0][2], g[1][3] - g[0][3]
                if (si - g[-1][2] == di and sj - g[-1][3] == dj
                        and len(frees) > len(g)):
                    g.append((i, j, si, sj))
                    continue
            groups.append([(i, j, si, sj)])
        for g in groups:
            cnt = len(g)
            ok = True
            if cnt >= 2:
                di, dj = g[1][2] - g[0][2], g[1][3] - g[0][3]
                ok = all(g[q][2] - g[q - 1][2] == di and
                         g[q][3] - g[q - 1][3] == dj
                         for q in range(1, cnt)) and len(frees) >= cnt
            if not ok:
                for (i, j, si, sj) in g:
                    fslot = frees.pop(0)
                    nc.vector.tensor_tensor(
                        out=ap2(fslot, 0, 1), in0=ap2(si, 0, 1),
                        in1=ap2(sj, 0, 1), op=mybir.AluOpType.max)
                    nc.vector.tensor_tensor(
                        out=ap2(si, 0, 1), in0=ap2(si, 0, 1),
                        in1=ap2(sj, 0, 1), op=mybir.AluOpType.min)
                    slot[j] = fslot
                    frees.append(sj)
                continue
            if cnt == 1:
                di = dj = 0
            fsl = [frees.pop(0) for _ in range(cnt)]
            fsl.sort()
            fo = fsl[1] - fsl[0] if cnt >= 2 else 0
            if cnt >= 2 and any(fsl[q] - fsl[q - 1] != fo
                                for q in range(1, cnt)):
                for (idx, (i, j, si, sj)) in enumerate(g):
                    fslot = fsl[idx]
                    nc.vector.tensor_tensor(
                        out=ap2(fslot, 0, 1), in0=ap2(si, 0, 1),
                        in1=ap2(sj, 0, 1), op=mybir.AluOpType.max)
                    nc.vector.tensor_tensor(
                        out=ap2(si, 0, 1), in0=ap2(si, 0, 1),
                        in1=ap2(sj, 0, 1), op=mybir.AluOpType.min)
                    slot[j] = fslot
                    frees.append(sj)
                continue
            s0, j0 = g[0][2], g[0][3]
            nc.vector.tensor_tensor(
                out=ap2(fsl[0], fo, cnt), in0=ap2(s0, di, cnt),
                in1=ap2(j0, dj, cnt), op=mybir.AluOpType.max)
            nc.vector.tensor_tensor(
                out=ap2(s0, di, cnt), in0=ap2(s0, di, cnt),
                in1=ap2(j0, dj, cnt), op=mybir.AluOpType.min)
            for (idx, (i, j, si, sj)) in enumerate(g):
                slot[j] = fsl[0] + idx * fo
                frees.append(sj)

    for k in range(emitted, len(placement)):
        placement[k][1]()

    # D_j = X_(19-j) - X_(j) into contiguous blocks of dtile
    for j in range(ND):
        nc.vector.tensor_tensor(
            out=dtile[:, j * fb:(j + 1) * fb],
            in0=ap2(slot[19 - j], 0, 1), in1=ap2(slot[j], 0, 1),
            op=mybir.AluOpType.subtract)
    # prescale by w_j = 19 - 2j (in place; plain tensor_scalar, 4x mode)
    for j in range(ND):
        nc.vector.tensor_scalar_mul(
            dtile[:, j * fb:(j + 1) * fb],
            dtile[:, j * fb:(j + 1) * fb],
            float(19 - 2 * j))
    # halve the ScalarE colsum: pre-add block pairs on DVE (one flat op)
    nc.vector.tensor_tensor(
        out=dtile[:, :5 * fb], in0=dtile[:, :5 * fb],
        in1=dtile[:, 5 * fb:ND * fb], op=mybir.AluOpType.add)
    # one ScalarE accumulate over the 5 pre-reduced blocks
    nc.scalar.activation(
        dtile[:, :5 * fb], dtile[:, :5 * fb],
        mybir.ActivationFunctionType.Copy,
        accum_out=acc[:, acc_col:acc_col + 1])


def _emit_pe_chunk_load(nc, pools, pex, chunk):
    """Phase 1: f16 DMA for one PE chunk (host pre-rounds to f16)."""
    import concourse.mybir as mybir
    from concourse.ap import AP

    F16 = mybir.dt.float16
    sp, pp = pools

    x16 = sp.tile([126, CHUNK_COLS], F16, tag="pex16")
    src = AP(pex[:, :].tensor, chunk * CHUNK_COLS,
             [[PE_COLS, 126], [1, CHUNK_COLS]])
    nc.sync.dma_start(x16, src)
    return x16


def _emit_pe_chunk_compute(nc, pools, wm, acc, x16, full, col0):
    """Phase 2: matmul passes + ScalarE Abs accumulate."""
    import concourse.mybir as mybir

    F32 = mybir.dt.float32
    F16 = mybir.dt.float16
    Abs = mybir.ActivationFunctionType.Abs
    sp, pp = pools

    passes = range(NPASS) if full else range(1)
    for p in passes:
        pt = pp.tile([128, CHUNK_COLS], F32, tag="pt")
        for st in range(CHUNK_COLS // MMF):
            nc.tensor.matmul(
                pt[:, st * MMF:(st + 1) * MMF],
                wm[:, p * 128:(p + 1) * 128],
                x16[:, st * MMF:(st + 1) * MMF],
                start=True, stop=True)
        # Abs in place on PSUM (ScalarE is closest to PSUM); the useful
        # result is the accumulator column.
        nc.scalar.activation(pt, pt, Abs,
                             accum_out=acc[:, col0 + p:col0 + p + 1])


def _build_nc():
    import concourse.bacc as bacc
    import concourse.mybir as mybir
    from concourse.tile import TileContext

    F32 = mybir.dt.float32
    F16 = mybir.dt.float16

    nc = bacc.Bacc()
    fsort = nc.declare_dram_parameter("fsort", [N, SORT_PX], F16,
                                      isOutput=False)
    pex = nc.declare_dram_parameter("pex", [126, PE_COLS], F16,
                                    isOutput=False)
    wmat = nc.declare_dram_parameter("wmat", [126, NPASS * 128], F16,
                                     isOutput=False)
    out = nc.declare_dram_parameter("acc", [128, NCOLS_PAD], F32,
                                    isOutput=True)

    with TileContext(nc) as tc:
        with (
            tc.tile_pool(name="wk", bufs=2) as wkp,
            tc.tile_pool(name="dp", bufs=1) as dpp,
            tc.tile_pool(name="pex16", bufs=12) as sp,
            tc.psum_pool(name="ps", bufs=2) as pp,
            tc.tile_pool(name="wmp", bufs=1) as wmp,
            tc.tile_pool(name="accp", bufs=1) as accp,
        ):
            wm = wmp.tile([126, NPASS * 128], F16)
            nc.sync.dma_start(wm, wmat[:, :])
            acc = accp.tile([128, NCOLS_PAD], F32)
            nc.vector.memset(acc[:, :], 0.0)

            sort_pools = (wkp, dpp)
            pe_pools = (sp, pp)

            # processing order: interleave full chunks (3 y : 1 full)
            y_chunks = list(range(TOTAL_CHUNKS - FULL_CHUNKS))
            full_chunks = list(range(TOTAL_CHUNKS - FULL_CHUNKS,
                                     TOTAL_CHUNKS))
            order = []
            yi = fi = 0
            for k in range(TOTAL_CHUNKS):
                if k % 3 == 2 and fi < len(full_chunks):
                    order.append((full_chunks[fi], True))
                    fi += 1
                else:
                    if yi < len(y_chunks):
                        order.append((y_chunks[yi], False))
                        yi += 1
                    else:
                        order.append((full_chunks[fi], True))
                        fi += 1

            # software pipeline: a couple of chunks up front (so ScalarE has
            # work during the first sort tile's DMA+L1), then each round's
            # remaining chunks emitted BETWEEN network layers (their casts
            # land inside the DVE stream, keeping PE/ACT fed), and one full
            # chunk held to the very end (ACT drains it while DVE finishes
            # the last network).
            rounds = len(SORT_FBS)
            ci = 0
            p0 = 0
            cols = {"y": N_SORT_COLS + N_FULL_COLS, "full": N_SORT_COLS}

            def make_emitter(chunk, full):
                col = cols["full" if full else "y"]
                cols["full" if full else "y"] += NPASS if full else 1

                def emit():
                    x16 = _emit_pe_chunk_load(nc, pe_pools, pex, chunk)
                    _emit_pe_chunk_compute(nc, pe_pools, wm, acc,
                                           x16, full, col)
                return emit

            # move one full chunk to the front (ACT runway during the first
            # sort tile's DMA+L1)
            first_full = next(i for i, (c, f) in enumerate(order) if f)
            order.insert(0, order.pop(first_full))
            emitters = [make_emitter(c, f) for (c, f) in order]
            weights = [NPASS if f else 1 for (c, f) in order]
            cum_fb = 0
            for t, fb in enumerate(SORT_FBS):
                cum_fb += fb
                n_here = (len(order) * cum_fb) // sum(SORT_FBS) - ci
                batch = list(range(ci, ci + n_here))
                ci += n_here
                if t == 0:
                    emitters[batch[0]]()
                    batch = batch[1:]
                buf, slot = _emit_sort_tile_load(nc, sort_pools, fsort,
                                                 p0, fb)
                p0 += 128 * fb
                _emit_sort_tile_sort(nc, sort_pools, buf, slot, acc, t, fb,
                                     [(emitters[i], weights[i])
                                      for i in batch])

            nc.sync.dma_start(out[:, :], acc[:, :])
    nc.compile()
    return nc


def _prep_core_inputs(fcf, tgf, c):
    """Build per-core input dict from full [N, P_TOTAL] / [P_TOTAL] arrays.

    Inputs are pre-rounded to f16 on the host (pure dtype cast, no math):
    the device computes in f16 either way, and rounding commutes exactly
    with min/max, so results match the previous on-device cast while
    halving HBM traffic and dropping the DVE convert passes.
    """
    sl = slice(c * P_CORE, (c + 1) * P_CORE)
    fcore = fcf[:, sl]
    tcore = tgf[sl]
    fsort = fcore[:, :SORT_PX].astype(np.float16)
    xs = fcore.reshape(N, TOTAL_CHUNKS, G, CHUNK_COLS)
    pex = np.empty((126, PE_COLS), np.float16)
    pex[0:120] = (xs.transpose(2, 0, 1, 3)
                  .reshape(G * N, PE_COLS))
    pex[120:126] = (tcore.reshape(TOTAL_CHUNKS, G, CHUNK_COLS)
                    .transpose(1, 0, 2).reshape(G, PE_COLS))
    return {"fsort": fsort, "pex": pex, "wmat": _CACHE["wmat"]}


def _combine(acc_list):
    y_total = 0.0
    pw_total = 0.0
    for acc in acc_list:
        a = np.asarray(acc, dtype=np.float64)
        pw_total += a[:, 0:N_SORT_COLS].sum()
        full = a[:, N_SORT_COLS:N_SORT_COLS + N_FULL_COLS].reshape(
            128, FULL_CHUNKS, NPASS)
        y_total += full[:, :, 0].sum()
        pw_total += full[:, :, 1:].sum()
        y_total += a[:, N_SORT_COLS + N_FULL_COLS:NCOLS].sum()
    val = (y_total / N - pw_total / (N * N)) / P_TOTAL
    return val


def _run(forecasts, target, trace=False):
    from concourse.bass_utils import run_bass_kernel_spmd

    nc = _CACHE.get("nc")
    if nc is None:
        _CACHE["wmat"] = build_wmat()
        nc = _build_nc()
        _CACHE["nc"] = nc

    fcf = np.asarray(forecasts, dtype=np.float32).reshape(N, P_TOTAL)
    tgf = np.asarray(target, dtype=np.float32).reshape(P_TOTAL)
    in_maps = [_prep_core_inputs(fcf, tgf, c) for c in range(N_CORES)]
    res = run_bass_kernel_spmd(nc, in_maps, list(range(N_CORES)), trace=trace)
    val = _combine([r["acc"] for r in res.results])
    return np.array(val, dtype=np.float32), res


def kernel(forecasts, target):
    val, _ = _run(forecasts, target)
    return val



# revision 54
# speedup vs baseline: 1.1905x; 1.1905x over previous
"""Trainium2 Bass kernel for empirical CRPS loss (mean reduction), v7.

All inputs are pre-rounded to f16 on the host (pure dtype cast -- rounding
commutes with min/max so results are identical to casting on device, and it
halves HBM traffic and eliminates the DVE convert passes).

Two concurrent pipelines per core (pure data parallel over pixels, 8 cores):

SORT path (pixels-on-free, [128, FB] f16 blocks), for SORT_PX pixels:
  per-pixel pairwise term Sum_{i<j}|x_i-x_j| = Sum_k (2k-19) X_(k) via a
  93-comparator sorting network on DVE (f16 2x mode, one DMA per tile,
  strided multi-block fusion), then the antisymmetric pairing
  D_j = X_(19-j) - X_(j) (10 DVE subs), prescale by w_j = 19-2j (DVE
  tensor_scalar, 4x mode), a DVE pre-reduce (10 -> 5 blocks), and ONE
  ScalarE accumulate -> 1 column of per-partition partial sums per tile.

PE path (samples-on-partitions [126, cols]): host pre-packs pe_x so rows
  20g+i = sample i of pixel-group g (6 groups), rows 120+g = target y.
  - ALL pixels: |x_i - y| term: 1 matmul pass (120 pair rows) per 512-col
    slice into PSUM + ScalarE Abs (in place on PSUM) + accum.
  - The last FULL_CHUNKS chunks additionally run 9 more passes covering all
    190 x-pairs per group (1140 rows) -> full pairwise term on PE+ACT
    instead of the DVE sort.
  Host separates x-pair vs y-pair contributions by pass column.

Scheduling: a small warmup sort tile first (DVE starts ~4us in); each
round's PE chunks are emitted BETWEEN network layers, placed proportionally
to their ScalarE cost with a half-step-early bias, so ScalarE never starves
while the network runs.

Host combine (f64): CRPS = (Y/N - (PW_sort + PW_pe)/N^2) / P_TOTAL.
"""
import numpy as np

N = 20
P_TOTAL = 4 * 1 * 12 * 256 * 256   # 3145728
N_CORES = 8
P_CORE = P_TOTAL // N_CORES        # 393216

G = 6                  # pixel groups in PE layout
CHUNK_COLS = 2048      # pe_x columns per chunk
CHUNK_PX = G * CHUNK_COLS          # 12288 px per chunk
TOTAL_CHUNKS = P_CORE // CHUNK_PX  # 32
FULL_CHUNKS = 10       # chunks whose pairwise term runs on PE+ACT
SORT_PX = P_CORE - FULL_CHUNKS * CHUNK_PX
PE_COLS = P_CORE // G  # 65536
NPASS = 10             # pass 0 = y pairs, 1..9 = x pairs
MMF = 512              # matmul free columns (one PSUM bank)

# sort tiling: SORT_PX/128 columns split into tiles; a small warmup tile
# first so DVE starts within ~4us instead of waiting on a 6MB DMA
SORT_FBS = [192, 960, 960]
assert sum(SORT_FBS) * 128 == SORT_PX
ND = 10                # D blocks

N_SORT_COLS = len(SORT_FBS)
N_FULL_COLS = FULL_CHUNKS * NPASS
N_Y_COLS = TOTAL_CHUNKS - FULL_CHUNKS
NCOLS = N_SORT_COLS + N_FULL_COLS + N_Y_COLS
NCOLS_PAD = ((NCOLS + 15) // 16) * 16

PAIRS = [(i, j) for i in range(N) for j in range(i + 1, N)]  # 190

_CACHE = {}

# --- sorting network (93 CEs): two optimal 29-CE 10-sorters + Batcher merge
SORT10 = [(4, 9), (3, 8), (2, 7), (1, 6), (0, 5),
          (1, 4), (6, 9), (0, 3), (5, 8),
          (0, 2), (3, 6), (7, 9),
          (0, 1), (2, 4), (5, 7), (8, 9),
          (1, 2), (4, 6), (7, 8), (3, 5),
          (2, 5), (6, 8), (1, 3), (4, 7),
          (2, 3), (6, 7),
          (3, 4), (5, 6),
          (4, 5)]


def _oe_merge(a, b, net):
    n, m = len(a), len(b)
    if n == 0 or m == 0:
        return
    if n == 1 and m == 1:
        net.append((a[0], b[0]))
        return
    _oe_merge(a[::2], b[::2], net)
    _oe_merge(a[1::2], b[1::2], net)
    c = list(a) + list(b)
    for i in range(1, n + m - 1, 2):
        net.append((c[i], c[i + 1]))


def sorting_network(n=N):
    assert n == 20
    net = [(i, j) for (i, j) in SORT10]
    net += [(i + 10, j + 10) for (i, j) in SORT10]
    _oe_merge(list(range(10)), list(range(10, 20)), net)
    return net


def build_wmat():
    w = np.zeros((126, NPASS * 128), np.float16)
    for g in range(G):                       # pass 0: y pairs
        for i in range(N):
            r = g * N + i
            w[20 * g + i, r] = 1.0
            w[120 + g, r] = -1.0
    rows = []
    for g in range(G):                       # passes 1..9: x pairs
        for (i, j) in PAIRS:
            rows.append((20 * g + i, 20 * g + j))
    for r, (pi, pj) in enumerate(rows):
        w[pi, 128 + r] = 1.0
        w[pj, 128 + r] = -1.0
    return w


def _emit_sort_tile_load(nc, pools, fsort, p0, fb):
    """One f16 DMA of all 20 sample blocks into buf; returns buf, slot.
    Inputs arrive pre-rounded to f16 from the host (rounding commutes with
    min/max, so the sorted values are bit-identical to sorting f32 then
    rounding)."""
    import concourse.mybir as mybir
    from concourse.ap import AP

    F16 = mybir.dt.float16
    wkp, dpp = pools
    NBUF = N + 4
    mfb = max(SORT_FBS)

    buf = wkp.tile([128, NBUF * mfb], F16, tag="buf")
    slot = list(range(N))

    src = AP(fsort[:, :].tensor, p0,
             [[fb, 128], [SORT_PX, N], [1, fb]])
    nc.sync.dma_start(
        buf[:, :N * fb].rearrange("p (n f) -> p n f", n=N), src)
    return buf, slot


def _emit_sort_tile_sort(nc, pools, buf, slot, acc, acc_col, fb,
                         chunk_emitters=()):
    """DVE sorting network + D blocks + prescale + pre-reduce + ScalarE
    colsum. `slot` holds the wire->block map from the load. Each entry of
    `chunk_emitters` is an (emit_fn, scalar_weight) pair emitted between
    network layers, placed by cumulative ScalarE cost, so ScalarE stays
    fed while the network runs on DVE."""
    import concourse.mybir as mybir
    from concourse.ap import AP

    F16 = mybir.dt.float16
    wkp, dpp = pools
    net = sorting_network(N)

    dtile = dpp.tile([128, ND * max(SORT_FBS)], F16, tag="d")

    # sort the 20 blocks (max -> free slot, min -> in place), fusing
    # wire-disjoint comparators with arithmetic-progression slots.
    frees = [N + k for k in range(4)]

    def ap2(base_slot, step, cnt):
        bap = buf[:, base_slot * fb:(base_slot + 1) * fb]
        if cnt == 1:
            return bap
        return AP(bap.tensor, bap.offset,
                  [list(bap.ap[0]), [step * fb, cnt], list(bap.ap[1])])

    layers, cur, used = [], [], set()
    for (i, j) in net:
        if i in used or j in used:
            layers.append(cur)
            cur, used = [], set()
        cur.append((i, j))
        used.update((i, j))
    layers.append(cur)

    # place chunk emissions across the network proportionally to their
    # ScalarE work (full chunk ~10 passes, y chunk 1) so ACT never starves
    chunk_emitters = list(chunk_emitters)
    nlay = len(layers)
    placement = []
    if chunk_emitters:
        tot_w = sum(w for _, w in chunk_emitters)
        cum = 0
        for em, w in chunk_emitters:
            # half-step early so ScalarE always has backlog
            lay = max(0, (2 * cum - w) * nlay // (2 * tot_w))
            placement.append((min(nlay - 1, lay), em))
            cum += w
    emitted = 0

    for li, layer in enumerate(layers):
        while emitted < len(placement) and placement[emitted][0] <= li:
            placement[emitted][1]()
            emitted += 1
        groups = []
        for (i, j) in layer:
            si, sj = slot[i], slot[j]
            g = groups[-1] if groups else None
            if g is not None and len(g) >= 1:
                if len(g) == 1:
                    g.append((i, j, si, sj))
                    continue
                di, dj = g[1][2] - g[0][2], g[1][3] - g[0][3]
                if (si - g[-1][2] == di and sj - g[-1][3] == dj
                        and len(frees) > len(g)):
                    g.append((i, j, si, sj))
                    continue
            groups.append([(i, j, si, sj)])
        for g in groups:
            cnt = len(g)
            ok = True
            if cnt >= 2:
                di, dj = g[1][2] - g[0][2], g[1][3] - g[0][3]
                ok = all(g[q][2] - g[q - 1][2] == di and
                         g[q][3] - g[q - 1][3] == dj
                         for q in range(1, cnt)) and len(frees) >= cnt
            if not ok:
                for (i, j, si, sj) in g:
                    fslot = frees.pop(0)
                    nc.vector.tensor_tensor(
                        out=ap2(fslot, 0, 1), in0=ap2(si, 0, 1),
                        in1=ap2(sj, 0, 1), op=mybir.AluOpType.max)
                    nc.vector.tensor_tensor(
                        out=ap2(si, 0, 1), in0=ap2(si, 0, 1),
                        in1=ap2(sj, 0, 1), op=mybir.AluOpType.min)
                    slot[j] = fslot
                    frees.append(sj)
                continue
            if cnt == 1:
                di = dj = 0
            fsl = [frees.pop(0) for _ in range(cnt)]
            fsl.sort()
            fo = fsl[1] - fsl[0] if cnt >= 2 else 0
            if cnt >= 2 and any(fsl[q] - fsl[q - 1] != fo
                                for q in range(1, cnt)):
                for (idx, (i, j, si, sj)) in enumerate(g):
                    fslot = fsl[idx]
                    nc.vector.tensor_tensor(
                        out=ap2(fslot, 0, 1), in0=ap2(si, 0, 1),
                        in1=ap2(sj, 0, 1), op=mybir.AluOpType.max)
                    nc.vector.tensor_tensor(
                        out=ap2(si, 0, 1), in0=ap2(si, 0, 1),
                        in1=ap2(sj, 0, 1), op=mybir.AluOpType.min)
                    slot[j] = fslot
                    frees.append(sj)
                continue
            s0, j0 = g[0][2], g[0][3]
            nc.vector.tensor_tensor(
                out=ap2(fsl[0], fo, cnt), in0=ap2(s0, di, cnt),
                in1=ap2(j0, dj, cnt), op=mybir.AluOpType.max)
            nc.vector.tensor_tensor(
                out=ap2(s0, di, cnt), in0=ap2(s0, di, cnt),
                in1=ap2(j0, dj, cnt), op=mybir.AluOpType.min)
            for (idx, (i, j, si, sj)) in enumerate(g):
                slot[j] = fsl[0] + idx * fo
                frees.append(sj)

    for k in range(emitted, len(placement)):
        placement[k][1]()

    # D_j = X_(19-j) - X_(j) into contiguous blocks of dtile
    for j in range(ND):
        nc.vector.tensor_tensor(
            out=dtile[:, j * fb:(j + 1) * fb],
            in0=ap2(slot[19 - j], 0, 1), in1=ap2(slot[j], 0, 1),
            op=mybir.AluOpType.subtract)
    # prescale by w_j = 19 - 2j (in place; plain tensor_scalar, 4x mode)
    for j in range(ND):
        nc.vector.tensor_scalar_mul(
            dtile[:, j * fb:(j + 1) * fb],
            dtile[:, j * fb:(j + 1) * fb],
            float(19 - 2 * j))
    # halve the ScalarE colsum: pre-add block pairs on DVE (one flat op)
    nc.vector.tensor_tensor(
        out=dtile[:, :5 * fb], in0=dtile[:, :5 * fb],
        in1=dtile[:, 5 * fb:ND * fb], op=mybir.AluOpType.add)
    # one ScalarE accumulate over the 5 pre-reduced blocks
    nc.scalar.activation(
        dtile[:, :5 * fb], dtile[:, :5 * fb],
        mybir.ActivationFunctionType.Copy,
        accum_out=acc[:, acc_col:acc_col + 1])


def _emit_pe_chunk_load(nc, pools, pex, chunk):
    """Phase 1: f16 DMA for one PE chunk (host pre-rounds to f16)."""
    import concourse.mybir as mybir
    from concourse.ap import AP

    F16 = mybir.dt.float16
    sp, pp = pools

    x16 = sp.tile([126, CHUNK_COLS], F16, tag="pex16")
    src = AP(pex[:, :].tensor, chunk * CHUNK_COLS,
             [[PE_COLS, 126], [1, CHUNK_COLS]])
    nc.sync.dma_start(x16, src)
    return x16


def _emit_pe_chunk_compute(nc, pools, wm, acc, x16, full, col0):
    """Phase 2: matmul passes + ScalarE Abs accumulate."""
    import concourse.mybir as mybir

    F32 = mybir.dt.float32
    F16 = mybir.dt.float16
    Abs = mybir.ActivationFunctionType.Abs
    sp, pp = pools

    passes = range(NPASS) if full else range(1)
    for p in passes:
        pt = pp.tile([128, CHUNK_COLS], F32, tag="pt")
        for st in range(CHUNK_COLS // MMF):
            nc.tensor.matmul(
                pt[:, st * MMF:(st + 1) * MMF],
                wm[:, p * 128:(p + 1) * 128],
                x16[:, st * MMF:(st + 1) * MMF],
                start=True, stop=True)
        # Abs in place on PSUM (ScalarE is closest to PSUM); the useful
        # result is the accumulator column.
        nc.scalar.activation(pt, pt, Abs,
                             accum_out=acc[:, col0 + p:col0 + p + 1])


def _build_nc():
    import concourse.bacc as bacc
    import concourse.mybir as mybir
    from concourse.tile import TileContext

    F32 = mybir.dt.float32
    F16 = mybir.dt.float16

    nc = bacc.Bacc()
    fsort = nc.declare_dram_parameter("fsort", [N, SORT_PX], F16,
                                      isOutput=False)
    pex = nc.declare_dram_parameter("pex", [126, PE_COLS], F16,
                                    isOutput=False)
    wmat = nc.declare_dram_parameter("wmat", [126, NPASS * 128], F16,
                                     isOutput=False)
    out = nc.declare_dram_parameter("acc", [128, NCOLS_PAD], F32,
                                    isOutput=True)

    with TileContext(nc) as tc:
        with (
            tc.tile_pool(name="wk", bufs=2) as wkp,
            tc.tile_pool(name="dp", bufs=1) as dpp,
            tc.tile_pool(name="pex16", bufs=12) as sp,
            tc.psum_pool(name="ps", bufs=2) as pp,
            tc.tile_pool(name="wmp", bufs=1) as wmp,
            tc.tile_pool(name="accp", bufs=1) as accp,
        ):
            wm = wmp.tile([126, NPASS * 128], F16)
            nc.sync.dma_start(wm, wmat[:, :])
            acc = accp.tile([128, NCOLS_PAD], F32)
            nc.vector.memset(acc[:, :], 0.0)

            sort_pools = (wkp, dpp)
            pe_pools = (sp, pp)

            # processing order: interleave full chunks (3 y : 1 full)
            y_chunks = list(range(TOTAL_CHUNKS - FULL_CHUNKS))
            full_chunks = list(range(TOTAL_CHUNKS - FULL_CHUNKS,
                                     TOTAL_CHUNKS))
            order = []
            yi = fi = 0
            for k in range(TOTAL_CHUNKS):
                if k % 3 == 2 and fi < len(full_chunks):
                    order.append((full_chunks[fi], True))
                    fi += 1
                else:
                    if yi < len(y_chunks):
                        order.append((y_chunks[yi], False))
                        yi += 1
                    else:
                        order.append((full_chunks[fi], True))
                        fi += 1

            # software pipeline: a couple of chunks up front (so ScalarE has
            # work during the first sort tile's DMA+L1), then each round's
            # remaining chunks emitted BETWEEN network layers (their casts
            # land inside the DVE stream, keeping PE/ACT fed), and one full
            # chunk held to the very end (ACT drains it while DVE finishes
            # the last network).
            rounds = len(SORT_FBS)
            ci = 0
            p0 = 0
            cols = {"y": N_SORT_COLS + N_FULL_COLS, "full": N_SORT_COLS}

            def make_emitter(chunk, full):
                col = cols["full" if full else "y"]
                cols["full" if full else "y"] += NPASS if full else 1

                def emit():
                    x16 = _emit_pe_chunk_load(nc, pe_pools, pex, chunk)
                    _emit_pe_chunk_compute(nc, pe_pools, wm, acc,
                                           x16, full, col)
                return emit

            # move one full chunk to the front (ACT runway during the first
            # sort tile's DMA+L1)
            first_full = next(i for i, (c, f) in enumerate(order) if f)
            order.insert(0, order.pop(first_full))
            emitters = [make_emitter(c, f) for (c, f) in order]
            weights = [NPASS if f else 1 for (c, f) in order]
            cum_fb = 0
            for t, fb in enumerate(SORT_FBS):
                cum_fb += fb
                n_here = (len(order) * cum_fb) // sum(SORT_FBS) - ci
                batch = list(range(ci, ci + n_here))
                ci += n_here
                if t == 0:
                    emitters[batch[0]]()
                    batch = batch[1:]
                buf, slot = _emit_sort_tile_load(nc, sort_pools, fsort,
                                                 p0, fb)
                p0 += 128 * fb
                _emit_sort_tile_sort(nc, sort_pools, buf, slot, acc, t, fb,
                                     [(emitters[i], weights[i])
                                      for i in batch])

            nc.sync.dma_start(out[:, :], acc[:, :])
    nc.compile()
    return nc


def _prep_core_inputs(fcf, tgf, c):
    """Build per-core input dict from full [N, P_TOTAL] / [P_TOTAL] arrays.

    Inputs are pre-rounded to f16 on the host (pure dtype cast, no math):
    the device computes in f16 either way, and rounding commutes exactly
    with min/max, so results match the previous on-device cast while
    halving HBM traffic and dropping the DVE convert passes.
    """
    sl = slice(c * P_CORE, (c + 1) * P_CORE)
    fcore = fcf[:, sl]
    tcore = tgf[sl]
    fsort = fcore[:, :SORT_PX].astype(np.float16)
    xs = fcore.reshape(N, TOTAL_CHUNKS, G, CHUNK_COLS)
    pex = np.empty((126, PE_COLS), np.float16)
    pex[0:120] = (xs.transpose(2, 0, 1, 3)
                  .reshape(G * N, PE_COLS))
    pex[120:126] = (tcore.reshape(TOTAL_CHUNKS, G, CHUNK_COLS)
                    .transpose(1, 0, 2).reshape(G, PE_COLS))
    return {"fsort": fsort, "pex": pex, "wmat": _CACHE["wmat"]}


def _combine(acc_list):
    y_total = 0.0
    pw_total = 0.0
    for acc in acc_list:
        a = np.asarray(acc, dtype=np.float64)
        pw_total += a[:, 0:N_SORT_COLS].sum()
        full = a[:, N_SORT_COLS:N_SORT_COLS + N_FULL_COLS].reshape(
            128, FULL_CHUNKS, NPASS)
        y_total += full[:, :, 0].sum()
        pw_total += full[:, :, 1:].sum()
        y_total += a[:, N_SORT_COLS + N_FULL_COLS:NCOLS].sum()
    val = (y_total / N - pw_total / (N * N)) / P_TOTAL
    return val


def _run(forecasts, target, trace=False):
    from concourse.bass_utils import run_bass_kernel_spmd

    nc = _CACHE.get("nc")
    if nc is None:
        _CACHE["wmat"] = build_wmat()
        nc = _build_nc()
        _CACHE["nc"] = nc

    fcf = np.asarray(forecasts, dtype=np.float32).reshape(N, P_TOTAL)
    tgf = np.asarray(target, dtype=np.float32).reshape(P_TOTAL)
    in_maps = [_prep_core_inputs(fcf, tgf, c) for c in range(N_CORES)]
    res = run_bass_kernel_spmd(nc, in_maps, list(range(N_CORES)), trace=trace)
    val = _combine([r["acc"] for r in res.results])
    return np.array(val, dtype=np.float32), res


def kernel(forecasts, target):
    val, _ = _run(forecasts, target)
    return val



# revision 60
# speedup vs baseline: 1.2019x; 1.0096x over previous
"""Trainium2 Bass kernel for empirical CRPS loss (mean reduction), v7.

All inputs are pre-rounded to f16 on the host (pure dtype cast -- rounding
commutes with min/max so results are identical to casting on device, and it
halves HBM traffic and eliminates the DVE convert passes).

Two concurrent pipelines per core (pure data parallel over pixels, 8 cores):

SORT path (pixels-on-free, [128, FB] f16 blocks), for SORT_PX pixels:
  per-pixel pairwise term Sum_{i<j}|x_i-x_j| = Sum_k (2k-19) X_(k) via a
  93-comparator sorting network on DVE (f16 2x mode, one DMA per tile,
  strided multi-block fusion), then the antisymmetric pairing
  D_j = X_(19-j) - X_(j) (10 DVE subs), prescale by w_j = 19-2j (DVE
  tensor_scalar, 4x mode), a DVE pre-reduce (10 -> 5 blocks), and ONE
  ScalarE accumulate -> 1 column of per-partition partial sums per tile.

PE path (samples-on-partitions [126, cols]): host pre-packs pe_x so rows
  20g+i = sample i of pixel-group g (6 groups), rows 120+g = target y.
  - ALL pixels: |x_i - y| term: 1 matmul pass (120 pair rows) per 512-col
    slice into PSUM + ScalarE Abs (in place on PSUM) + accum.
  - The last FULL_CHUNKS chunks additionally run 9 more passes covering all
    190 x-pairs per group (1140 rows) -> full pairwise term on PE+ACT
    instead of the DVE sort.
  Host separates x-pair vs y-pair contributions by pass column.

Scheduling: a small warmup sort tile first (DVE starts ~4us in); each
round's PE chunks are emitted BETWEEN network layers, placed proportionally
to their ScalarE cost with a half-step-early bias, so ScalarE never starves
while the network runs.

Host combine (f64): CRPS = (Y/N - (PW_sort + PW_pe)/N^2) / P_TOTAL.
"""
import numpy as np

N = 20
P_TOTAL = 4 * 1 * 12 * 256 * 256   # 3145728
N_CORES = 8
P_CORE = P_TOTAL // N_CORES        # 393216

G = 6                  # pixel groups in PE layout
CHUNK_COLS = 2048      # pe_x columns per chunk
CHUNK_PX = G * CHUNK_COLS          # 12288 px per chunk
TOTAL_CHUNKS = P_CORE // CHUNK_PX  # 32
FULL_CHUNKS = 10       # chunks whose pairwise term runs on PE+ACT
SORT_PX = P_CORE - FULL_CHUNKS * CHUNK_PX
PE_COLS = P_CORE // G  # 65536
NPASS = 10             # pass 0 = y pairs, 1..9 = x pairs
MMF = 512              # matmul free columns (one PSUM bank)

# sort tiling: SORT_PX/128 columns split into tiles; a small warmup tile
# first so DVE starts within ~4us instead of waiting on a 6MB DMA
SORT_FBS = [192, 960, 960]
PRERED = 5             # colsum width after DVE pre-reduce (5 or 3)
TTR_TAIL = False       # fuse D/prescale/colsum into chained tensor_tensor_reduce
EARLY = 1              # chunk placement bias, in half-steps of its weight
SP_BUFS = 12           # x16 staging buffers
assert sum(SORT_FBS) * 128 == SORT_PX
ND = 10                # D blocks

N_SORT_COLS = len(SORT_FBS)
N_FULL_COLS = FULL_CHUNKS * NPASS
N_Y_COLS = TOTAL_CHUNKS - FULL_CHUNKS
NCOLS = N_SORT_COLS + N_FULL_COLS + N_Y_COLS
NCOLS_PAD = ((NCOLS + 15) // 16) * 16

PAIRS = [(i, j) for i in range(N) for j in range(i + 1, N)]  # 190

_CACHE = {}

# --- sorting network (93 CEs): two optimal 29-CE 10-sorters + Batcher merge
SORT10 = [(4, 9), (3, 8), (2, 7), (1, 6), (0, 5),
          (1, 4), (6, 9), (0, 3), (5, 8),
          (0, 2), (3, 6), (7, 9),
          (0, 1), (2, 4), (5, 7), (8, 9),
          (1, 2), (4, 6), (7, 8), (3, 5),
          (2, 5), (6, 8), (1, 3), (4, 7),
          (2, 3), (6, 7),
          (3, 4), (5, 6),
          (4, 5)]


def _oe_merge(a, b, net):
    n, m = len(a), len(b)
    if n == 0 or m == 0:
        return
    if n == 1 and m == 1:
        net.append((a[0], b[0]))
        return
    _oe_merge(a[::2], b[::2], net)
    _oe_merge(a[1::2], b[1::2], net)
    c = list(a) + list(b)
    for i in range(1, n + m - 1, 2):
        net.append((c[i], c[i + 1]))


def sorting_network(n=N):
    assert n == 20
    net = [(i, j) for (i, j) in SORT10]
    net += [(i + 10, j + 10) for (i, j) in SORT10]
    _oe_merge(list(range(10)), list(range(10, 20)), net)
    return net


def build_wmat():
    w = np.zeros((126, NPASS * 128), np.float16)
    for g in range(G):                       # pass 0: y pairs
        for i in range(N):
            r = g * N + i
            w[20 * g + i, r] = 1.0
            w[120 + g, r] = -1.0
    rows = []
    for g in range(G):                       # passes 1..9: x pairs
        for (i, j) in PAIRS:
            rows.append((20 * g + i, 20 * g + j))
    for r, (pi, pj) in enumerate(rows):
        w[pi, 128 + r] = 1.0
        w[pj, 128 + r] = -1.0
    return w


def _emit_sort_tile_load(nc, pools, fsort, p0, fb):
    """One f16 DMA of all 20 sample blocks into buf; returns buf, slot.
    Inputs arrive pre-rounded to f16 from the host (rounding commutes with
    min/max, so the sorted values are bit-identical to sorting f32 then
    rounding)."""
    import concourse.mybir as mybir
    from concourse.ap import AP

    F16 = mybir.dt.float16
    wkp, dpp = pools
    NBUF = N + 4
    mfb = max(SORT_FBS)

    buf = wkp.tile([128, NBUF * mfb], F16, tag="buf")
    slot = list(range(N))

    src = AP(fsort[:, :].tensor, p0,
             [[fb, 128], [SORT_PX, N], [1, fb]])
    nc.sync.dma_start(
        buf[:, :N * fb].rearrange("p (n f) -> p n f", n=N), src)
    return buf, slot


def _emit_sort_tile_sort(nc, pools, buf, slot, acc, acc_col, fb,
                         chunk_emitters=()):
    """DVE sorting network + D blocks + prescale + pre-reduce + ScalarE
    colsum. `slot` holds the wire->block map from the load. Each entry of
    `chunk_emitters` is an (emit_fn, scalar_weight) pair emitted between
    network layers, placed by cumulative ScalarE cost, so ScalarE stays
    fed while the network runs on DVE."""
    import concourse.mybir as mybir
    from concourse.ap import AP

    F16 = mybir.dt.float16
    wkp, dpp = pools
    net = sorting_network(N)

    dtile = dpp.tile([128, ND * max(SORT_FBS)], F16, tag="d")

    # sort the 20 blocks (max -> free slot, min -> in place), fusing
    # wire-disjoint comparators with arithmetic-progression slots.
    frees = [N + k for k in range(4)]

    def ap2(base_slot, step, cnt):
        bap = buf[:, base_slot * fb:(base_slot + 1) * fb]
        if cnt == 1:
            return bap
        return AP(bap.tensor, bap.offset,
                  [list(bap.ap[0]), [step * fb, cnt], list(bap.ap[1])])

    layers, cur, used = [], [], set()
    for (i, j) in net:
        if i in used or j in used:
            layers.append(cur)
            cur, used = [], set()
        cur.append((i, j))
        used.update((i, j))
    layers.append(cur)

    # place chunk emissions across the network proportionally to their
    # ScalarE work (full chunk ~10 passes, y chunk 1) so ACT never starves
    chunk_emitters = list(chunk_emitters)
    nlay = len(layers)
    placement = []
    if chunk_emitters:
        tot_w = sum(w for _, w in chunk_emitters)
        cum = 0
        for em, w in chunk_emitters:
            # biased early so ScalarE always has backlog
            lay = max(0, (2 * cum - EARLY * w) * nlay // (2 * tot_w))
            placement.append((min(nlay - 1, lay), em))
            cum += w
    emitted = 0

    for li, layer in enumerate(layers):
        while emitted < len(placement) and placement[emitted][0] <= li:
            placement[emitted][1]()
            emitted += 1
        groups = []
        for (i, j) in layer:
            si, sj = slot[i], slot[j]
            g = groups[-1] if groups else None
            if g is not None and len(g) >= 1:
                if len(g) == 1:
                    g.append((i, j, si, sj))
                    continue
                di, dj = g[1][2] - g[0][2], g[1][3] - g[0][3]
                if (si - g[-1][2] == di and sj - g[-1][3] == dj
                        and len(frees) > len(g)):
                    g.append((i, j, si, sj))
                    continue
            groups.append([(i, j, si, sj)])
        for g in groups:
            cnt = len(g)
            ok = True
            if cnt >= 2:
                di, dj = g[1][2] - g[0][2], g[1][3] - g[0][3]
                ok = all(g[q][2] - g[q - 1][2] == di and
                         g[q][3] - g[q - 1][3] == dj
                         for q in range(1, cnt)) and len(frees) >= cnt
            if not ok:
                for (i, j, si, sj) in g:
                    fslot = frees.pop(0)
                    nc.vector.tensor_tensor(
                        out=ap2(fslot, 0, 1), in0=ap2(si, 0, 1),
                        in1=ap2(sj, 0, 1), op=mybir.AluOpType.max)
                    nc.vector.tensor_tensor(
                        out=ap2(si, 0, 1), in0=ap2(si, 0, 1),
                        in1=ap2(sj, 0, 1), op=mybir.AluOpType.min)
                    slot[j] = fslot
                    frees.append(sj)
                continue
            if cnt == 1:
                di = dj = 0
            fsl = [frees.pop(0) for _ in range(cnt)]
            fsl.sort()
            fo = fsl[1] - fsl[0] if cnt >= 2 else 0
            if cnt >= 2 and any(fsl[q] - fsl[q - 1] != fo
                                for q in range(1, cnt)):
                for (idx, (i, j, si, sj)) in enumerate(g):
                    fslot = fsl[idx]
                    nc.vector.tensor_tensor(
                        out=ap2(fslot, 0, 1), in0=ap2(si, 0, 1),
                        in1=ap2(sj, 0, 1), op=mybir.AluOpType.max)
                    nc.vector.tensor_tensor(
                        out=ap2(si, 0, 1), in0=ap2(si, 0, 1),
                        in1=ap2(sj, 0, 1), op=mybir.AluOpType.min)
                    slot[j] = fslot
                    frees.append(sj)
                continue
            s0, j0 = g[0][2], g[0][3]
            nc.vector.tensor_tensor(
                out=ap2(fsl[0], fo, cnt), in0=ap2(s0, di, cnt),
                in1=ap2(j0, dj, cnt), op=mybir.AluOpType.max)
            nc.vector.tensor_tensor(
                out=ap2(s0, di, cnt), in0=ap2(s0, di, cnt),
                in1=ap2(j0, dj, cnt), op=mybir.AluOpType.min)
            for (idx, (i, j, si, sj)) in enumerate(g):
                slot[j] = fsl[0] + idx * fo
                frees.append(sj)

    for k in range(emitted, len(placement)):
        placement[k][1]()

    if TTR_TAIL:
        # fused: out = (X_(19-j) - X_(j)) * w_j, running free-dim sum
        # chained through the acc column -- no ScalarE colsum at all
        col = acc[:, acc_col:acc_col + 1]
        for j in range(ND):
            nc.vector.tensor_tensor_reduce(
                out=dtile[:, j * fb:(j + 1) * fb],
                in0=ap2(slot[19 - j], 0, 1), in1=ap2(slot[j], 0, 1),
                scale=float(19 - 2 * j),
                scalar=(0.0 if j == 0 else col),
                op0=mybir.AluOpType.subtract,
                op1=mybir.AluOpType.add,
                accum_out=col)
        return

    # D_j = X_(19-j) - X_(j) into contiguous blocks of dtile
    for j in range(ND):
        nc.vector.tensor_tensor(
            out=dtile[:, j * fb:(j + 1) * fb],
            in0=ap2(slot[19 - j], 0, 1), in1=ap2(slot[j], 0, 1),
            op=mybir.AluOpType.subtract)
    # prescale by w_j = 19 - 2j (in place; plain tensor_scalar, 4x mode)
    for j in range(ND):
        nc.vector.tensor_scalar_mul(
            dtile[:, j * fb:(j + 1) * fb],
            dtile[:, j * fb:(j + 1) * fb],
            float(19 - 2 * j))
    # shrink the ScalarE colsum: pre-add block pairs on DVE (flat ops)
    nc.vector.tensor_tensor(
        out=dtile[:, :5 * fb], in0=dtile[:, :5 * fb],
        in1=dtile[:, 5 * fb:ND * fb], op=mybir.AluOpType.add)
    if PRERED == 3:
        nc.vector.tensor_tensor(
            out=dtile[:, :2 * fb], in0=dtile[:, :2 * fb],
            in1=dtile[:, 3 * fb:5 * fb], op=mybir.AluOpType.add)
    # one ScalarE accumulate over the pre-reduced blocks
    nc.scalar.activation(
        dtile[:, :PRERED * fb], dtile[:, :PRERED * fb],
        mybir.ActivationFunctionType.Copy,
        accum_out=acc[:, acc_col:acc_col + 1])


def _emit_pe_chunk_load(nc, pools, pex, chunk):
    """Phase 1: f16 DMA for one PE chunk (host pre-rounds to f16)."""
    import concourse.mybir as mybir
    from concourse.ap import AP

    F16 = mybir.dt.float16
    sp, pp = pools

    x16 = sp.tile([126, CHUNK_COLS], F16, tag="pex16")
    src = AP(pex[:, :].tensor, chunk * CHUNK_COLS,
             [[PE_COLS, 126], [1, CHUNK_COLS]])
    nc.sync.dma_start(x16, src)
    return x16


def _emit_pe_chunk_compute(nc, pools, wm, acc, x16, full, col0):
    """Phase 2: matmul passes + ScalarE Abs accumulate."""
    import concourse.mybir as mybir

    F32 = mybir.dt.float32
    F16 = mybir.dt.float16
    Abs = mybir.ActivationFunctionType.Abs
    sp, pp = pools

    passes = range(NPASS) if full else range(1)
    for p in passes:
        pt = pp.tile([128, CHUNK_COLS], F32, tag="pt")
        for st in range(CHUNK_COLS // MMF):
            nc.tensor.matmul(
                pt[:, st * MMF:(st + 1) * MMF],
                wm[:, p * 128:(p + 1) * 128],
                x16[:, st * MMF:(st + 1) * MMF],
                start=True, stop=True)
        # Abs in place on PSUM (ScalarE is closest to PSUM); the useful
        # result is the accumulator column.
        nc.scalar.activation(pt, pt, Abs,
                             accum_out=acc[:, col0 + p:col0 + p + 1])


def _build_nc():
    import concourse.bacc as bacc
    import concourse.mybir as mybir
    from concourse.tile import TileContext

    F32 = mybir.dt.float32
    F16 = mybir.dt.float16

    nc = bacc.Bacc()
    fsort = nc.declare_dram_parameter("fsort", [N, SORT_PX], F16,
                                      isOutput=False)
    pex = nc.declare_dram_parameter("pex", [126, PE_COLS], F16,
                                    isOutput=False)
    wmat = nc.declare_dram_parameter("wmat", [126, NPASS * 128], F16,
                                     isOutput=False)
    out = nc.declare_dram_parameter("acc", [128, NCOLS_PAD], F32,
                                    isOutput=True)

    with TileContext(nc) as tc:
        with (
            tc.tile_pool(name="wk", bufs=2) as wkp,
            tc.tile_pool(name="dp", bufs=1) as dpp,
            tc.tile_pool(name="pex16", bufs=SP_BUFS) as sp,
            tc.psum_pool(name="ps", bufs=2) as pp,
            tc.tile_pool(name="wmp", bufs=1) as wmp,
            tc.tile_pool(name="accp", bufs=1) as accp,
        ):
            wm = wmp.tile([126, NPASS * 128], F16)
            nc.sync.dma_start(wm, wmat[:, :])
            acc = accp.tile([128, NCOLS_PAD], F32)
            nc.vector.memset(acc[:, :], 0.0)

            sort_pools = (wkp, dpp)
            pe_pools = (sp, pp)

            # processing order: interleave full chunks (3 y : 1 full)
            y_chunks = list(range(TOTAL_CHUNKS - FULL_CHUNKS))
            full_chunks = list(range(TOTAL_CHUNKS - FULL_CHUNKS,
                                     TOTAL_CHUNKS))
            order = []
            yi = fi = 0
            for k in range(TOTAL_CHUNKS):
                if k % 3 == 2 and fi < len(full_chunks):
                    order.append((full_chunks[fi], True))
                    fi += 1
                else:
                    if yi < len(y_chunks):
                        order.append((y_chunks[yi], False))
                        yi += 1
                    else:
                        order.append((full_chunks[fi], True))
                        fi += 1

            # software pipeline: a couple of chunks up front (so ScalarE has
            # work during the first sort tile's DMA+L1), then each round's
            # remaining chunks emitted BETWEEN network layers (their casts
            # land inside the DVE stream, keeping PE/ACT fed), and one full
            # chunk held to the very end (ACT drains it while DVE finishes
            # the last network).
            rounds = len(SORT_FBS)
            ci = 0
            p0 = 0
            cols = {"y": N_SORT_COLS + N_FULL_COLS, "full": N_SORT_COLS}

            def make_emitter(chunk, full):
                col = cols["full" if full else "y"]
                cols["full" if full else "y"] += NPASS if full else 1

                def emit():
                    x16 = _emit_pe_chunk_load(nc, pe_pools, pex, chunk)
                    _emit_pe_chunk_compute(nc, pe_pools, wm, acc,
                                           x16, full, col)
                return emit

            # move one full chunk to the front (ACT runway during the first
            # sort tile's DMA+L1)
            first_full = next(i for i, (c, f) in enumerate(order) if f)
            order.insert(0, order.pop(first_full))
            emitters = [make_emitter(c, f) for (c, f) in order]
            weights = [NPASS if f else 1 for (c, f) in order]
            cum_fb = 0
            for t, fb in enumerate(SORT_FBS):
                cum_fb += fb
                n_here = (len(order) * cum_fb) // sum(SORT_FBS) - ci
                batch = list(range(ci, ci + n_here))
                ci += n_here
                if t == 0:
                    emitters[batch[0]]()
                    batch = batch[1:]
                buf, slot = _emit_sort_tile_load(nc, sort_pools, fsort,
                                                 p0, fb)
                p0 += 128 * fb
                _emit_sort_tile_sort(nc, sort_pools, buf, slot, acc, t, fb,
                                     [(emitters[i], weights[i])
                                      for i in batch])

            nc.sync.dma_start(out[:, :], acc[:, :])
    nc.compile()
    return nc


def _prep_core_inputs(fcf, tgf, c):
    """Build per-core input dict from full [N, P_TOTAL] / [P_TOTAL] arrays.

    Inputs are pre-rounded to f16 on the host (pure dtype cast, no math):
    the device computes in f16 either way, and rounding commutes exactly
    with min/max, so results match the previous on-device cast while
    halving HBM traffic and dropping the DVE convert passes.
    """
    sl = slice(c * P_CORE, (c + 1) * P_CORE)
    fcore = fcf[:, sl]
    tcore = tgf[sl]
    fsort = fcore[:, :SORT_PX].astype(np.float16)
    xs = fcore.reshape(N, TOTAL_CHUNKS, G, CHUNK_COLS)
    pex = np.empty((126, PE_COLS), np.float16)
    pex[0:120] = (xs.transpose(2, 0, 1, 3)
                  .reshape(G * N, PE_COLS))
    pex[120:126] = (tcore.reshape(TOTAL_CHUNKS, G, CHUNK_COLS)
                    .transpose(1, 0, 2).reshape(G, PE_COLS))
    return {"fsort": fsort, "pex": pex, "wmat": _CACHE["wmat"]}


def _combine(acc_list):
    y_total = 0.0
    pw_total = 0.0
    for acc in acc_list:
        a = np.asarray(acc, dtype=np.float64)
        pw_total += a[:, 0:N_SORT_COLS].sum()
        full = a[:, N_SORT_COLS:N_SORT_COLS + N_FULL_COLS].reshape(
            128, FULL_CHUNKS, NPASS)
        y_total += full[:, :, 0].sum()
        pw_total += full[:, :, 1:].sum()
        y_total += a[:, N_SORT_COLS + N_FULL_COLS:NCOLS].sum()
    val = (y_total / N - pw_total / (N * N)) / P_TOTAL
    return val


def _run(forecasts, target, trace=False):
    from concourse.bass_utils import run_bass_kernel_spmd

    nc = _CACHE.get("nc")
    if nc is None:
        _CACHE["wmat"] = build_wmat()
        nc = _build_nc()
        _CACHE["nc"] = nc

    fcf = np.asarray(forecasts, dtype=np.float32).reshape(N, P_TOTAL)
    tgf = np.asarray(target, dtype=np.float32).reshape(P_TOTAL)
    in_maps = [_prep_core_inputs(fcf, tgf, c) for c in range(N_CORES)]
    res = run_bass_kernel_spmd(nc, in_maps, list(range(N_CORES)), trace=trace)
    val = _combine([r["acc"] for r in res.results])
    return np.array(val, dtype=np.float32), res


def kernel(forecasts, target):
    val, _ = _run(forecasts, target)
    return val

